# revision 16
# baseline (speedup 1.0000x reference)
"""Distributed Bass kernel for nn_Attention (B=8, S=1024, H=768, nh=12).

Sharding: data-parallel over batch — core b computes batch element b.
No collectives; host side shards, layout-permutes, and pre-folds inputs.

Key restructure vs the v1 baseline (3.42 ms -> ~0.65 ms marginal HW):
  host folds mask+bias into  A[i,k,q] = (1-mask)·exp(bias)  so the device
  computes  p = A ⊙ exp(s) + mask  with
  - exp(s) read directly from PSUM on ACT (no separate bias add pass),
  - A-mul / mask-add split across Pool and DVE (both all-bf16 SBUF),
  - projection biases folded into PE accumulation via ones-row matmuls,
  - all bulk loads as single mega-DMAs through rearranged 3D access
    patterns (~30 DMAs total vs ~290),
  - deep PSUM buffering (pss bufs=3) — measured worth ~200us on HW,
  - mask/h loads on the SWDGE ring so the SP HWDGE ring starts on
    weights immediately.

Per-core pipeline (bf16 matmuls, f32 PSUM):
  QT = SCALE·(h@Wq+bq).T stored [j, s]; KT likewise unscaled.
  VZ[st] = [128, 12·65] tiles: per head 64 V columns + a ones column
           (so attn@V also yields the softmax denominator Z).
  per head i, kt: ps = K_tile^T Q (PSUM); e = exp(ps) (ACT);
                  t = e·A_tile; p = t + maskT (Pool/DVE).
  po[0:64] = V^T p accumulated over kt; po[64] = Z.
  OUTT = po[0:64] · bcast(1/Z)   (ones-row matmul broadcast)
  res[s, j] = OUTT^T @ Wo + bo (bias via ones-row matmul), DMA out.
"""
import sys
import functools
import numpy as np

sys.path.insert(0, "/opt/trn_rl_repo")

NH, D, S, H, P = 12, 64, 1024, 768, 128
NT = H // P          # 6 chunks of the hidden dim
ST = S // P          # 8 tiles of the sequence dim
SCALE = D ** -0.5    # 0.125


def _body(nc, tc, tile, mybir, dr, out_dram):
    f32 = mybir.dt.float32
    bf16 = mybir.dt.bfloat16
    AF = mybir.ActivationFunctionType
    ALU = mybir.AluOpType
    from concourse import bass
    PSUM = bass.MemorySpace.PSUM

    with (
        tc.tile_pool(name="qt", bufs=1) as qt_pool,
        tc.tile_pool(name="kt", bufs=1) as kt_pool,
        tc.tile_pool(name="vz", bufs=1) as vz_pool,
        tc.tile_pool(name="mt", bufs=1) as mt_pool,
        tc.tile_pool(name="ot", bufs=1) as ot_pool,
        tc.tile_pool(name="cst", bufs=1) as cst_pool,
        # phase-2/3 pools hoisted out of the phase-1 scope so their SBUF
        # addresses never alias phase-1 pools: A/Wo prefetch DMAs and head-0
        # exp/mul can then overlap the tail of phase 1 instead of waiting on
        # address-reuse WAR deps.
        tc.tile_pool(name="apool", bufs=2) as a_pool,
        tc.tile_pool(name="wo", bufs=1) as wo_pool,
        tc.tile_pool(name="pt", bufs=2) as pt_pool,
        tc.tile_pool(name="esc", bufs=4) as e_pool,
        tc.tile_pool(name="tsc", bufs=4) as t_pool,
        tc.tile_pool(name="rz", bufs=2) as rz_pool,
    ):
        QT = [qt_pool.tile([P, S], bf16, name=f"QT{t}") for t in range(NT)]
        KT = [kt_pool.tile([P, S], bf16, name=f"KT{t}") for t in range(NT)]
        VZ = [vz_pool.tile([P, NH * (D + 1)], bf16, name=f"VZ{t}") for t in range(ST)]
        MTB = mt_pool.tile([P, ST * S], bf16, name="MTB")
        MT = [MTB[:, kt * S : (kt + 1) * S] for kt in range(ST)]
        OUTT = [ot_pool.tile([P, S], bf16, name=f"OUTT{t}") for t in range(NT)]
        ones_row = cst_pool.tile([1, P], bf16, name="ones_row")
        bqs = cst_pool.tile([P, NT], f32, name="bqs")
        bks = cst_pool.tile([P, NT], f32, name="bks")
        bvr = cst_pool.tile([1, H], bf16, name="bvr")
        bor = cst_pool.tile([1, H], bf16, name="bor")

        nc.vector.memset(ones_row[:], 1.0)
        # bq/bk as [128, 6] partition-major; pre-scale bq by SCALE
        nc.sync.dma_start(bqs[:], dr["bq"].rearrange("(t p) -> p t", p=P))
        nc.sync.dma_start(bks[:], dr["bk"].rearrange("(t p) -> p t", p=P))
        nc.vector.tensor_scalar_mul(bqs[:], bqs[:], float(SCALE))

        ABIG = [None] * NH

        def issue_a(i):
            # same name for every head -> one pool tag, heads rotate its 2 bufs
            ABIG[i] = a_pool.tile([P, ST * S], bf16, name="abig")
            nc.sync.dma_start(
                ABIG[i][:], dr["A"][i].rearrange("(kt p) q -> p kt q", p=P)
            )

        # ---------------- phase 1: projections ----------------
        with (
            tc.tile_pool(name="hp", bufs=1) as hp_pool,
            tc.tile_pool(name="wp", bufs=1) as wp_pool,
            tc.tile_pool(name="psp", bufs=6, space=PSUM) as psp_pool,
        ):
            hTB = hp_pool.tile([P, NT * S], bf16, name="hTB")
            nc.gpsimd.dma_start(hTB[:], dr["hT"].rearrange("(c p) s -> p c s", p=P))
            # mask.T behind hTB on the SWDGE ring (not needed until phase 2);
            # bvr/bor behind the Q/K weights on SP (needed for V-proj/phase 3)
            nc.gpsimd.dma_start(
                MTB[:], dr["maskT"].rearrange("(kt p) q -> p kt q", p=P)
            )
            hT = [hTB[:, c * S : (c + 1) * S] for c in range(NT)]
            wqB = wp_pool.tile([P, NT * H], bf16, name="wqB")
            wkB = wp_pool.tile([P, NT * H], bf16, name="wkB")
            wvB = wp_pool.tile([P, NT * H], bf16, name="wvB")
            nc.sync.dma_start(wqB[:], dr["Wq"].rearrange("(c p) j -> p c j", p=P))
            nc.sync.dma_start(wkB[:], dr["Wk"].rearrange("(c p) j -> p c j", p=P))
            nc.gpsimd.dma_start(wvB[:], dr["Wv"].rearrange("(c p) j -> p c j", p=P))
            nc.sync.dma_start(bvr[:], dr["bv"][:])
            nc.sync.dma_start(bor[:], dr["bo"][:])
            # prefetch first two heads' A and the Wo weights on the SP ring
            # behind the projection weights — they land well before use.
            issue_a(0)
            issue_a(1)
            woB = wo_pool.tile([P, NT * H], bf16, name="woB")
            nc.sync.dma_start(woB[:], dr["Wo"].rearrange("(c p) j -> p c j", p=P))
            wq = [wqB[:, c * H : (c + 1) * H] for c in range(NT)]
            wk = [wkB[:, c * H : (c + 1) * H] for c in range(NT)]
            wv = [wvB[:, c * H : (c + 1) * H] for c in range(NT)]

            # QT / KT: [j, s] layout; bias+scale on ACT (bqs pre-scaled)
            for wlist, dst, s1, btile in (
                (wq, QT, float(SCALE), bqs),
                (wk, KT, 1.0, bks),
            ):
                for t in range(NT):
                    for sc in range(2):
                        ps = psp_pool.tile([P, 512], f32, name="psp")
                        for c in range(NT):
                            nc.tensor.matmul(
                                ps[:],
                                wlist[c][:, t * P : (t + 1) * P],
                                hT[c][:, sc * 512 : (sc + 1) * 512],
                                start=(c == 0),
                                stop=(c == NT - 1),
                            )
                        nc.scalar.activation(
                            dst[t][:, sc * 512 : (sc + 1) * 512],
                            ps[:],
                            AF.Identity,
                            bias=btile[:, t : t + 1],
                            scale=s1,
                        )

            # V -> VZ with ones column per head; bias via ones-row matmul
            for st in range(ST):
                nc.vector.memset(VZ[st][:], 1.0)
            for jc in range(2):
                for st in range(ST):
                    ps = psp_pool.tile([P, 512], f32, name="psp")
                    for c in range(NT):
                        nc.tensor.matmul(
                            ps[:, 0:384],
                            hT[c][:, st * P : (st + 1) * P],
                            wv[c][:, jc * 384 : (jc + 1) * 384],
                            start=(c == 0),
                            stop=False,
                        )
                    nc.tensor.matmul(
                        ps[:, 0:384],
                        ones_row[:],
                        bvr[0:1, jc * 384 : (jc + 1) * 384],
                        start=False,
                        stop=True,
                    )
                    for hh in range(6):
                        i = jc * 6 + hh
                        nc.vector.tensor_scalar_mul(
                            VZ[st][:, i * 65 : i * 65 + 64],
                            ps[:, hh * 64 : (hh + 1) * 64],
                            1.0,
                        )

        # ---------------- phase 2: attention per head ----------------
        with (
            tc.tile_pool(name="pss", bufs=3, space=PSUM) as pss_pool,
            tc.tile_pool(name="pso", bufs=2, space=PSUM) as pso_pool,
        ):
            for i in range(NH):
                ch, off = i // 2, (i % 2) * D
                abig = ABIG[i]
                pts = [pt_pool.tile([P, S], bf16, name=f"pt{kt}") for kt in range(ST)]
                for kt in range(ST):
                    ps = pss_pool.tile([P, S], f32, name="pss")
                    for qc in range(2):
                        nc.tensor.matmul(
                            ps[:, qc * 512 : (qc + 1) * 512],
                            KT[ch][off : off + D, kt * P : (kt + 1) * P],
                            QT[ch][off : off + D, qc * 512 : (qc + 1) * 512],
                            start=True,
                            stop=True,
                        )
                    e = e_pool.tile([P, S], bf16, name="e")
                    nc.scalar.activation(e[:], ps[:], AF.Exp)
                    t1 = t_pool.tile([P, S], bf16, name="t1")
                    # all-bf16 all-SBUF ops run in DVE 4x mode (~0.26ns/elem);
                    # Pool's sw Add/Multiply is ~7x slower — keep the bulk
                    # path entirely on DVE.
                    nc.vector.tensor_mul(
                        t1[:], e[:], abig[:, kt * S : (kt + 1) * S]
                    )
                    nc.vector.tensor_add(pts[kt][:], t1[:], MT[kt])
                # issue head i+2's A load only after head i's last abig read
                # (the slot it reuses is now free)
                if i + 2 < NH:
                    issue_a(i + 2)
                for qc in range(2):
                    po = pso_pool.tile([D + 1, 512], f32, name="pso")
                    for kt in range(ST):
                        nc.tensor.matmul(
                            po[:],
                            VZ[kt][:, i * 65 : (i + 1) * 65],
                            pts[kt][:, qc * 512 : (qc + 1) * 512],
                            start=(kt == 0),
                            stop=(kt == ST - 1),
                        )
                    rz = rz_pool.tile([1, 512], bf16, name="rz")
                    with nc.allow_low_precision(reason="1/Z in bf16 for bcast"):
                        nc.vector.reciprocal(rz[:], po[D : D + 1, :])
                    # replicate 1/Z across 64 partitions on Pool (GPSIMD
                    # extended inst; SBUF->SBUF so it's legal there), then a
                    # single DVE mul with po as the lone PSUM operand.
                    rzb = rz_pool.tile([D, 512], bf16, name="rzb")
                    nc.gpsimd.partition_broadcast(rzb[:], rz[:])
                    nc.vector.tensor_mul(
                        OUTT[ch][off : off + D, qc * 512 : (qc + 1) * 512],
                        po[0:D, :],
                        rzb[:],
                    )

        # ---------------- phase 3: output projection ----------------
        with (
            tc.tile_pool(name="res", bufs=4) as res_pool,
            tc.tile_pool(name="psr", bufs=4, space=PSUM) as psr_pool,
        ):
            wo = [woB[:, c * H : (c + 1) * H] for c in range(NT)]
            for st in range(ST):
                res = res_pool.tile([P, H], f32, name="res")
                for jc in range(2):
                    ps = psr_pool.tile([P, 512], f32, name="psr")
                    for ch in range(NT):
                        nc.tensor.matmul(
                            ps[:, 0:384],
                            OUTT[ch][:, st * P : (st + 1) * P],
                            wo[ch][:, jc * 384 : (jc + 1) * 384],
                            start=(ch == 0),
                            stop=False,
                        )
                    nc.tensor.matmul(
                        ps[:, 0:384],
                        ones_row[:],
                        bor[0:1, jc * 384 : (jc + 1) * 384],
                        start=False,
                        stop=True,
                    )
                    nc.vector.tensor_scalar_mul(
                        res[:, jc * 384 : (jc + 1) * 384],
                        ps[:, 0:384],
                        1.0,
                    )
                oeng = nc.sync if st % 2 == 1 else nc.gpsimd
                oeng.dma_start(out_dram[st * P : (st + 1) * P, :], res[:])


@functools.lru_cache(maxsize=1)
def _build():
    from concourse import bacc, tile, mybir

    nc = bacc.Bacc("TRN2", target_bir_lowering=False, debug=False, num_devices=8)
    f32 = mybir.dt.float32
    bf16 = mybir.dt.bfloat16
    dr = {
        "hT": nc.dram_tensor("hT", [H, S], bf16, kind="ExternalInput").ap(),
        "A": nc.dram_tensor("A", [NH, S, S], bf16, kind="ExternalInput").ap(),
        "maskT": nc.dram_tensor("maskT", [S, S], bf16, kind="ExternalInput").ap(),
    }
    for w in ("Wq", "Wk", "Wv", "Wo"):
        dr[w] = nc.dram_tensor(w, [H, H], bf16, kind="ExternalInput").ap()
    for b in ("bq", "bk"):
        dr[b] = nc.dram_tensor(b, [H], f32, kind="ExternalInput").ap()
    for b in ("bv", "bo"):
        dr[b] = nc.dram_tensor(b, [H], bf16, kind="ExternalInput").ap()
    out = nc.dram_tensor("out", [S, H], f32, kind="ExternalOutput").ap()

    with tile.TileContext(nc) as tc:
        _body(nc, tc, tile, mybir, dr, out)
    nc.compile()
    return nc


def make_in_maps(**inputs):
    import ml_dtypes
    bf = ml_dtypes.bfloat16
    h = np.asarray(inputs["h"], np.float32)
    ab = np.asarray(inputs["att_bias"], np.float32)
    mk = np.asarray(inputs["mask"], np.int32)
    shared = {
        "bq": np.asarray(inputs["bq"], np.float32),
        "bk": np.asarray(inputs["bk"], np.float32),
        "bv": np.asarray(inputs["bv"], np.float32).astype(bf),
        "bo": np.asarray(inputs["bo"], np.float32).astype(bf),
    }
    for k in ("Wq", "Wk", "Wv", "Wo"):
        shared[k] = np.asarray(inputs[k], np.float32).astype(bf)
    in_maps = []
    for b in range(8):
        m = dict(shared)
        m["hT"] = np.ascontiguousarray(h[b].T).astype(bf)
        # A[i,k,q] = (1-mask[q,k]) * exp(bias[q,k,i]); maskT[k,q]=mask[q,k]
        mb = mk[b].astype(np.float32)           # [q, k]
        a = np.exp(ab[b]) * (1.0 - mb)[:, :, None]
        m["A"] = np.ascontiguousarray(a.transpose(2, 1, 0)).astype(bf)
        m["maskT"] = np.ascontiguousarray(mb.T).astype(bf)
        in_maps.append(m)
    return in_maps


def kernel(**inputs):
    nc = _build()
    from concourse import bass_utils

    in_maps = make_in_maps(**inputs)
    res = bass_utils.run_bass_kernel_spmd(nc, in_maps, core_ids=list(range(8)))
    return np.stack([r["out"] for r in res.results], axis=0)



# revision 28
# speedup vs baseline: 1.0803x; 1.0803x over previous
"""Distributed Bass kernel for nn_Attention (B=8, S=1024, H=768, nh=12).

Sharding: data-parallel over batch — core b computes batch element b.
No collectives; host side shards, layout-permutes, and pre-folds inputs.

v7 structure — software-pipelined phases (sim ~175us vs 380us v1):
  host folds mask+bias into  A[i,k,q] = (1-mask)·exp(bias)  so the device
  computes  p = A ⊙ exp(s) + mask, normalized by the Z row that the
  VZ ones-column produces inside the attn@V accumulation.

  All tile pools (SBUF and PSUM) coexist for the whole kernel so no
  cross-phase address-reuse WAR ever serializes a phase boundary.
  PSUM budget: psp 2 banks | pss 2x2 banks | pso 2 banks = 8.

  Emission interleaves the projections into the head loop so every engine
  pipelines across phases:
    pre:  Q/K chunk t=0
    head i: scores(i) + exp/mul/add per kt; then at i==0 the remaining
            Q/K chunk t=1 and the whole V projection; at even i the Q/K
            chunk t=i//2+1 (feeds heads i+2, i+3); then attnV(i-1) and
            its 1/Z tail; A(i+2) DMA issued after head i's last A read.
    post: attnV(11) + tail, then the output projection.

  Engine allocation (per-head steady state):
    PE   scores 16mm + attnV 16mm + interleaved projection chunks
    ACT  8x exp [128,1024] + the Q/K bias+scale epilogues
    DVE  14 of 16 A-mul/+M ops (bf16 2x mode) + 1/Z tail muls + VZ copies
    Pool 2 of 16 bulk ops + partition_broadcast of 1/Z
    DMA  A loads: head 0 on the ACT ring, head 1 on the DVE ring, rest
         on the SP ring behind the weights; h/mask/Wv on the SWDGE ring.
"""
import sys
import functools
import numpy as np

sys.path.insert(0, "/opt/trn_rl_repo")

NH, D, S, H, P = 12, 64, 1024, 768, 128
NT = H // P          # 6 chunks of the hidden dim
ST = S // P          # 8 tiles of the sequence dim
SCALE = D ** -0.5    # 0.125


def _body(nc, tc, tile, mybir, dr, out_dram):
    f32 = mybir.dt.float32
    bf16 = mybir.dt.bfloat16
    AF = mybir.ActivationFunctionType
    ALU = mybir.AluOpType
    from concourse import bass
    PSUM = bass.MemorySpace.PSUM

    from contextlib import ExitStack

    with ExitStack() as stack:
        pool = lambda name, bufs, **kw: stack.enter_context(
            tc.tile_pool(name=name, bufs=bufs, **kw)
        )
        qt_pool = pool("qt", 1)
        kt_pool = pool("kt", 1)
        vz_pool = pool("vz", 1)
        mt_pool = pool("mt", 1)
        ot_pool = pool("ot", 1)
        cst_pool = pool("cst", 1)
        a_pool = pool("apool", 2)
        wo_pool = pool("wo", 1)
        pt_pool = pool("pt", 2)
        e_pool = pool("esc", 4)
        t_pool = pool("tsc", 4)
        rz_pool = pool("rz", 2)
        # pools released before the output projection (frees 39KB SBUF for
        # res and all 8 PSUM banks for psr)
        inner = stack.enter_context(ExitStack())
        ipool = lambda name, bufs, **kw: inner.enter_context(
            tc.tile_pool(name=name, bufs=bufs, **kw)
        )
        hp_pool = ipool("hp", 1)
        wp_pool = ipool("wp", 1)
        psp_pool = ipool("psp", 2, space=PSUM)
        pss_pool = ipool("pss", 2, space=PSUM)
        pso_pool = ipool("pso", 2, space=PSUM)
        QT = [qt_pool.tile([P, S], bf16, name=f"QT{t}") for t in range(NT)]
        KT = [kt_pool.tile([P, S], bf16, name=f"KT{t}") for t in range(NT)]
        VZ = [vz_pool.tile([P, NH * (D + 1)], bf16, name=f"VZ{t}") for t in range(ST)]
        MTB = mt_pool.tile([P, ST * S], bf16, name="MTB")
        MT = [MTB[:, kt * S : (kt + 1) * S] for kt in range(ST)]
        OUTT = [ot_pool.tile([P, S], bf16, name=f"OUTT{t}") for t in range(NT)]
        ones_row = cst_pool.tile([1, P], bf16, name="ones_row")
        bqs = cst_pool.tile([P, NT], f32, name="bqs")
        bks = cst_pool.tile([P, NT], f32, name="bks")
        bvr = cst_pool.tile([1, H], bf16, name="bvr")
        bor = cst_pool.tile([1, H], bf16, name="bor")

        nc.vector.memset(ones_row[:], 1.0)
        # bq/bk as [128, 6] partition-major; pre-scale bq by SCALE
        nc.sync.dma_start(bqs[:], dr["bq"].rearrange("(t p) -> p t", p=P))
        nc.sync.dma_start(bks[:], dr["bk"].rearrange("(t p) -> p t", p=P))
        nc.vector.tensor_scalar_mul(bqs[:], bqs[:], float(SCALE))

        ABIG = [None] * NH

        def issue_a(i, eng):
            # same name for every head -> one pool tag, heads rotate 2 bufs
            ABIG[i] = a_pool.tile([P, ST * S], bf16, name="abig")
            eng.dma_start(
                ABIG[i][:], dr["A"][i].rearrange("(kt p) q -> p kt q", p=P)
            )

        # A0/A1 on the otherwise-idle ACT HWDGE ring so they don't queue
        # behind the weights on SP (DVE has no HWDGE ring on this config)
        issue_a(0, nc.scalar)
        issue_a(1, nc.scalar)

        # bulk input loads: h/mask/Wv on the SWDGE ring, Q/K/O weights on SP
        hTB = hp_pool.tile([P, NT * S], bf16, name="hTB")
        nc.gpsimd.dma_start(hTB[:], dr["hT"].rearrange("(c p) s -> p c s", p=P))
        nc.gpsimd.dma_start(
            MTB[:], dr["maskT"].rearrange("(kt p) q -> p kt q", p=P)
        )
        hT = [hTB[:, c * S : (c + 1) * S] for c in range(NT)]
        wqB = wp_pool.tile([P, NT * H], bf16, name="wqB")
        wkB = wp_pool.tile([P, NT * H], bf16, name="wkB")
        wvB = wp_pool.tile([P, NT * H], bf16, name="wvB")
        nc.sync.dma_start(wqB[:], dr["Wq"].rearrange("(c p) j -> p c j", p=P))
        nc.sync.dma_start(wkB[:], dr["Wk"].rearrange("(c p) j -> p c j", p=P))
        nc.gpsimd.dma_start(wvB[:], dr["Wv"].rearrange("(c p) j -> p c j", p=P))
        nc.sync.dma_start(bvr[:], dr["bv"][:])
        nc.sync.dma_start(bor[:], dr["bo"][:])
        woB = wo_pool.tile([P, NT * H], bf16, name="woB")
        nc.sync.dma_start(woB[:], dr["Wo"].rearrange("(c p) j -> p c j", p=P))
        wq = [wqB[:, c * H : (c + 1) * H] for c in range(NT)]
        wk = [wkB[:, c * H : (c + 1) * H] for c in range(NT)]
        wv = [wvB[:, c * H : (c + 1) * H] for c in range(NT)]

        def qk_tile(t, w_idx, sc):
            # one PSUM tile of the Q or K projection for chunk t
            wlist, dst, s1, btile = (
                (wq, QT, float(SCALE), bqs),
                (wk, KT, 1.0, bks),
            )[w_idx]
            ps = psp_pool.tile([P, 512], f32, name="psp")
            for c in range(NT):
                nc.tensor.matmul(
                    ps[:],
                    wlist[c][:, t * P : (t + 1) * P],
                    hT[c][:, sc * 512 : (sc + 1) * 512],
                    start=(c == 0),
                    stop=(c == NT - 1),
                )
            nc.scalar.activation(
                dst[t][:, sc * 512 : (sc + 1) * 512],
                ps[:],
                AF.Identity,
                bias=btile[:, t : t + 1],
                scale=s1,
            )

        def qk_chunk(t):
            for w_idx in range(2):
                for sc in range(2):
                    qk_tile(t, w_idx, sc)

        def v_tile(jc, st):
            # one PSUM tile of the V projection (+bias via ones-row matmul)
            ps = psp_pool.tile([P, 512], f32, name="psp")
            for c in range(NT):
                nc.tensor.matmul(
                    ps[:, 0:384],
                    hT[c][:, st * P : (st + 1) * P],
                    wv[c][:, jc * 384 : (jc + 1) * 384],
                    start=(c == 0),
                    stop=False,
                )
            nc.tensor.matmul(
                ps[:, 0:384],
                ones_row[:],
                bvr[0:1, jc * 384 : (jc + 1) * 384],
                start=False,
                stop=True,
            )
            for hh in range(6):
                i = jc * 6 + hh
                nc.vector.tensor_scalar_mul(
                    VZ[st][:, i * 65 : i * 65 + 64],
                    ps[:, hh * 64 : (hh + 1) * 64],
                    1.0,
                )

        PTS = [None] * NH
        PO = [None] * NH

        def attnv_step(i, kt):
            # two matmuls of head i's attn@V (one per qc), interleaved into
            # the next head's score loop; accumulation groups for the two po
            # banks stay open across interleaved matmuls (skip_group_check)
            if kt == 0:
                PO[i] = [
                    pso_pool.tile([D + 1, 512], f32, name="pso")
                    for _ in range(2)
                ]
            pts = PTS[i]
            for qc in range(2):
                nc.tensor.matmul(
                    PO[i][qc][:],
                    VZ[kt][:, i * 65 : (i + 1) * 65],
                    pts[kt][:, qc * 512 : (qc + 1) * 512],
                    start=(kt == 0),
                    stop=(kt == ST - 1),
                    skip_group_check=True,
                )

        def z_tail(i):
            ch, off = i // 2, (i % 2) * D
            for qc in range(2):
                po = PO[i][qc]
                rz = rz_pool.tile([1, 512], bf16, name="rz")
                with nc.allow_low_precision(reason="1/Z in bf16 for bcast"):
                    nc.vector.reciprocal(rz[:], po[D : D + 1, :])
                # replicate 1/Z across 64 partitions on Pool (SBUF->SBUF),
                # then one DVE mul with po as the lone PSUM operand
                rzb = rz_pool.tile([D, 512], bf16, name="rzb")
                nc.gpsimd.partition_broadcast(rzb[:], rz[:])
                nc.vector.tensor_mul(
                    OUTT[ch][off : off + D, qc * 512 : (qc + 1) * 512],
                    po[0:D, :],
                    rzb[:],
                )

        # ---------------- pipelined projections + attention ----------------
        # VZ memsets up front (ones columns survive the per-tile writes)
        for st in range(ST):
            nc.vector.memset(VZ[st][:], 1.0)
        qk_chunk(0)
        for i in range(NH):
            ch, off = i // 2, (i % 2) * D
            abig = ABIG[i]
            pts = PTS[i] = [
                pt_pool.tile([P, S], bf16, name=f"pt{kt}") for kt in range(ST)
            ]
            for kt in range(ST):
                ps = pss_pool.tile([P, S], f32, name="pss")
                for qc in range(2):
                    nc.tensor.matmul(
                        ps[:, qc * 512 : (qc + 1) * 512],
                        KT[ch][off : off + D, kt * P : (kt + 1) * P],
                        QT[ch][off : off + D, qc * 512 : (qc + 1) * 512],
                        start=True,
                        stop=True,
                    )
                e = e_pool.tile([P, S], bf16, name="e")
                nc.scalar.activation(e[:], ps[:], AF.Exp)
                t1 = t_pool.tile([P, S], bf16, name="t1")
                # DVE runs bf16 tensor_tensor at 2x (~590ns/tile); Pool's
                # sw path is ~2.2us/tile. 2 of 16 ops per head go to Pool.
                if kt == 1:
                    nc.gpsimd.tensor_mul(
                        t1[:], e[:], abig[:, kt * S : (kt + 1) * S]
                    )
                else:
                    nc.vector.tensor_mul(
                        t1[:], e[:], abig[:, kt * S : (kt + 1) * S]
                    )
                if kt == 5:
                    nc.gpsimd.tensor_add(pts[kt][:], t1[:], MT[kt])
                else:
                    nc.vector.tensor_add(pts[kt][:], t1[:], MT[kt])
                # interleave projection tiles / previous head's attn@V into
                # this head's score loop so PE never batches a long block
                # between two heads' exp streams
                if i == 0:
                    v_tile(0, kt)
                elif i == 1:
                    v_tile(1, kt)
                elif i in (2, 4, 6, 8) and kt < 4:
                    qk_tile(i // 2 + 1, w_idx=kt // 2, sc=kt % 2)
                if i == 0 and kt >= 4:
                    qk_tile(1, w_idx=(kt - 4) // 2, sc=kt % 2)
                if i >= 1:
                    attnv_step(i - 1, kt)
            # head i+2's A load reuses head i's buffer slot — issue after
            # head i's last abig read
            if i + 2 < NH:
                issue_a(i + 2, nc.sync)
            if i >= 1:
                z_tail(i - 1)
        for kt in range(ST):
            attnv_step(NH - 1, kt)
        z_tail(NH - 1)
        inner.close()

        # ---------------- output projection ----------------
        with (
            tc.tile_pool(name="res", bufs=4) as res_pool,
            tc.tile_pool(name="psr", bufs=4, space=PSUM) as psr_pool,
        ):
            wo = [woB[:, c * H : (c + 1) * H] for c in range(NT)]
            for st in range(ST):
                res = res_pool.tile([P, H], f32, name="res")
                for jc in range(2):
                    ps = psr_pool.tile([P, 512], f32, name="psr")
                    for ch in range(NT):
                        nc.tensor.matmul(
                            ps[:, 0:384],
                            OUTT[ch][:, st * P : (st + 1) * P],
                            wo[ch][:, jc * 384 : (jc + 1) * 384],
                            start=(ch == 0),
                            stop=False,
                        )
                    nc.tensor.matmul(
                        ps[:, 0:384],
                        ones_row[:],
                        bor[0:1, jc * 384 : (jc + 1) * 384],
                        start=False,
                        stop=True,
                    )
                    nc.vector.tensor_scalar_mul(
                        res[:, jc * 384 : (jc + 1) * 384],
                        ps[:, 0:384],
                        1.0,
                    )
                oeng = nc.sync if st % 2 == 1 else nc.gpsimd
                oeng.dma_start(out_dram[st * P : (st + 1) * P, :], res[:])


@functools.lru_cache(maxsize=1)
def _build():
    from concourse import bacc, tile, mybir

    nc = bacc.Bacc("TRN2", target_bir_lowering=False, debug=False, num_devices=8)
    f32 = mybir.dt.float32
    bf16 = mybir.dt.bfloat16
    dr = {
        "hT": nc.dram_tensor("hT", [H, S], bf16, kind="ExternalInput").ap(),
        "A": nc.dram_tensor("A", [NH, S, S], bf16, kind="ExternalInput").ap(),
        "maskT": nc.dram_tensor("maskT", [S, S], bf16, kind="ExternalInput").ap(),
    }
    for w in ("Wq", "Wk", "Wv", "Wo"):
        dr[w] = nc.dram_tensor(w, [H, H], bf16, kind="ExternalInput").ap()
    for b in ("bq", "bk"):
        dr[b] = nc.dram_tensor(b, [H], f32, kind="ExternalInput").ap()
    for b in ("bv", "bo"):
        dr[b] = nc.dram_tensor(b, [H], bf16, kind="ExternalInput").ap()
    out = nc.dram_tensor("out", [S, H], f32, kind="ExternalOutput").ap()

    with tile.TileContext(nc) as tc:
        _body(nc, tc, tile, mybir, dr, out)
    nc.compile()
    return nc


def make_in_maps(**inputs):
    import ml_dtypes
    bf = ml_dtypes.bfloat16
    h = np.asarray(inputs["h"], np.float32)
    ab = np.asarray(inputs["att_bias"], np.float32)
    mk = np.asarray(inputs["mask"], np.int32)
    shared = {
        "bq": np.asarray(inputs["bq"], np.float32),
        "bk": np.asarray(inputs["bk"], np.float32),
        "bv": np.asarray(inputs["bv"], np.float32).astype(bf),
        "bo": np.asarray(inputs["bo"], np.float32).astype(bf),
    }
    for k in ("Wq", "Wk", "Wv", "Wo"):
        shared[k] = np.asarray(inputs[k], np.float32).astype(bf)
    in_maps = []
    for b in range(8):
        m = dict(shared)
        m["hT"] = np.ascontiguousarray(h[b].T).astype(bf)
        # A[i,k,q] = (1-mask[q,k]) * exp(bias[q,k,i]); maskT[k,q]=mask[q,k]
        mb = mk[b].astype(np.float32)           # [q, k]
        a = np.exp(ab[b]) * (1.0 - mb)[:, :, None]
        m["A"] = np.ascontiguousarray(a.transpose(2, 1, 0)).astype(bf)
        m["maskT"] = np.ascontiguousarray(mb.T).astype(bf)
        in_maps.append(m)
    return in_maps


def kernel(**inputs):
    nc = _build()
    from concourse import bass_utils

    in_maps = make_in_maps(**inputs)
    res = bass_utils.run_bass_kernel_spmd(nc, in_maps, core_ids=list(range(8)))
    return np.stack([r["out"] for r in res.results], axis=0)


# revision 40
# speedup vs baseline: 1.1474x; 1.0622x over previous
"""Distributed Bass kernel for nn_Attention (B=8, S=1024, H=768, nh=12).

Sharding: data-parallel over batch — core b computes batch element b.
No collectives; host side shards, layout-permutes, and pre-folds inputs.

v7 structure — software-pipelined phases (sim ~175us vs 380us v1):
  host folds mask+bias into  A[i,k,q] = (1-mask)·exp(bias)  so the device
  computes  p = A ⊙ exp(s) + mask, normalized by the Z row that the
  VZ ones-column produces inside the attn@V accumulation.

  All tile pools (SBUF and PSUM) coexist for the whole kernel so no
  cross-phase address-reuse WAR ever serializes a phase boundary.
  PSUM budget: psp 2 banks | pss 2x2 banks | pso 2 banks = 8.

  Emission interleaves the projections into the head loop so every engine
  pipelines across phases:
    pre:  Q/K chunk t=0
    head i: scores(i) + exp/mul/add per kt; then at i==0 the remaining
            Q/K chunk t=1 and the whole V projection; at even i the Q/K
            chunk t=i//2+1 (feeds heads i+2, i+3); then attnV(i-1) and
            its 1/Z tail; A(i+2) DMA issued after head i's last A read.
    post: attnV(11) + tail, then the output projection.

  Engine allocation (per-head steady state):
    PE   scores 16mm + attnV 16mm + interleaved projection chunks
    ACT  8x exp [128,1024] + the Q/K bias+scale epilogues
    DVE  14 of 16 A-mul/+M ops (bf16 2x mode) + 1/Z tail muls + VZ copies
    Pool 2 of 16 bulk ops + partition_broadcast of 1/Z
    DMA  A loads: head 0 on the ACT ring, head 1 on the DVE ring, rest
         on the SP ring behind the weights; h/mask/Wv on the SWDGE ring.
"""
import sys
import functools
import numpy as np

sys.path.insert(0, "/opt/trn_rl_repo")

NH, D, S, H, P = 12, 64, 1024, 768, 128
NT = H // P          # 6 chunks of the hidden dim
ST = S // P          # 8 tiles of the sequence dim
SCALE = D ** -0.5    # 0.125


def _body(nc, tc, tile, mybir, dr, out_dram):
    f32 = mybir.dt.float32
    bf16 = mybir.dt.bfloat16
    AF = mybir.ActivationFunctionType
    ALU = mybir.AluOpType
    from concourse import bass
    PSUM = bass.MemorySpace.PSUM

    from contextlib import ExitStack

    with ExitStack() as stack:
        pool = lambda name, bufs, **kw: stack.enter_context(
            tc.tile_pool(name=name, bufs=bufs, **kw)
        )
        qt_pool = pool("qt", 1)
        kt_pool = pool("kt", 1)
        vz_pool = pool("vz", 1)
        mt_pool = pool("mt", 1)
        ot_pool = pool("ot", 1)
        cst_pool = pool("cst", 1)
        a_pool = pool("apool", 2)
        wo_pool = pool("wo", 1)
        pt_pool = pool("pt", 2)
        e_pool = pool("esc", 4)
        t_pool = pool("tsc", 4)
        rz_pool = pool("rz", 2)
        # pools released before the output projection (frees 39KB SBUF for
        # res and all 8 PSUM banks for psr)
        inner = stack.enter_context(ExitStack())
        ipool = lambda name, bufs, **kw: inner.enter_context(
            tc.tile_pool(name=name, bufs=bufs, **kw)
        )
        hp_pool = ipool("hp", 1)
        wp_pool = ipool("wp", 1)
        psp_pool = ipool("psp", 2, space=PSUM)
        pss_pool = ipool("pss", 2, space=PSUM)
        pso_pool = ipool("pso", 2, space=PSUM)
        QT = [qt_pool.tile([P, S], bf16, name=f"QT{t}") for t in range(NT)]
        KT = [kt_pool.tile([P, S], bf16, name=f"KT{t}") for t in range(NT)]
        VZ = [vz_pool.tile([P, NH * (D + 1)], bf16, name=f"VZ{t}") for t in range(ST)]
        MTB = mt_pool.tile([P, ST * S], bf16, name="MTB")
        MT = [MTB[:, kt * S : (kt + 1) * S] for kt in range(ST)]
        OUTT = [ot_pool.tile([P, S], bf16, name=f"OUTT{t}") for t in range(NT)]
        bqs = cst_pool.tile([P, NT], f32, name="bqs")
        bks = cst_pool.tile([P, NT], f32, name="bks")
        bvr = cst_pool.tile([1, H], bf16, name="bvr")
        bor = cst_pool.tile([1, H], bf16, name="bor")

        bvrB = cst_pool.tile([P, H], bf16, name="bvrB")
        borB = cst_pool.tile([P, H], bf16, name="borB")


        ABIG = [None] * NH

        def issue_a(i, eng):
            # same name for every head -> one pool tag, heads rotate 2 bufs
            ABIG[i] = a_pool.tile([P, ST * S], bf16, name="abig")
            eng.dma_start(
                ABIG[i][:], dr["A"][i].rearrange("(kt p) q -> p kt q", p=P)
            )

        # A0/A1 on the otherwise-idle ACT HWDGE ring so they don't queue
        # behind the weights on SP (DVE has no HWDGE ring on this config)
        issue_a(0, nc.scalar)
        issue_a(1, nc.scalar)

        # bulk input loads: h/mask/Wv on the SWDGE ring, Q/K/O weights on SP
        hTB = hp_pool.tile([P, NT * S], bf16, name="hTB")
        nc.gpsimd.dma_start(hTB[:], dr["hT"].rearrange("(c p) s -> p c s", p=P))
        nc.gpsimd.dma_start(
            MTB[:], dr["maskT"].rearrange("(kt p) q -> p kt q", p=P)
        )
        hT = [hTB[:, c * S : (c + 1) * S] for c in range(NT)]
        wqB = wp_pool.tile([P, NT * H], bf16, name="wqB")
        wkB = wp_pool.tile([P, NT * H], bf16, name="wkB")
        wvB = wp_pool.tile([P, NT * H], bf16, name="wvB")
        # Wq/Wk first on SP: Kt0 (and so the whole exp stream) is gated on
        # wkB; the small consts follow.
        nc.sync.dma_start(wqB[:], dr["Wq"].rearrange("(c p) j -> p c j", p=P))
        nc.sync.dma_start(wkB[:], dr["Wk"].rearrange("(c p) j -> p c j", p=P))
        # bq/bk as [128, 6] partition-major; pre-scale bq by SCALE
        nc.sync.dma_start(bqs[:], dr["bq"].rearrange("(t p) -> p t", p=P))
        nc.sync.dma_start(bks[:], dr["bk"].rearrange("(t p) -> p t", p=P))
        nc.vector.tensor_scalar_mul(bqs[:], bqs[:], float(SCALE))
        nc.sync.dma_start(bvr[:], dr["bv"][:])
        nc.sync.dma_start(bor[:], dr["bo"][:])
        nc.gpsimd.dma_start(wvB[:], dr["Wv"].rearrange("(c p) j -> p c j", p=P))
        # bias rows broadcast across partitions once (after the SWDGE DMA
        # triggers so h/mask aren't delayed): the per-tile bias ones-row
        # matmuls in V-proj and the output projection become plain DVE adds
        nc.gpsimd.partition_broadcast(bvrB[:], bvr[:])
        nc.gpsimd.partition_broadcast(borB[:], bor[:])
        woB = wo_pool.tile([P, NT * H], bf16, name="woB")
        nc.sync.dma_start(woB[:], dr["Wo"].rearrange("(c p) j -> p c j", p=P))
        wq = [wqB[:, c * H : (c + 1) * H] for c in range(NT)]
        wk = [wkB[:, c * H : (c + 1) * H] for c in range(NT)]
        wv = [wvB[:, c * H : (c + 1) * H] for c in range(NT)]

        def qk_tile(t, w_idx, sc):
            # one PSUM tile of the Q or K projection for chunk t
            wlist, dst, s1, btile = (
                (wq, QT, float(SCALE), bqs),
                (wk, KT, 1.0, bks),
            )[w_idx]
            ps = psp_pool.tile([P, 512], f32, name="psp")
            for c in range(NT):
                nc.tensor.matmul(
                    ps[:],
                    wlist[c][:, t * P : (t + 1) * P],
                    hT[c][:, sc * 512 : (sc + 1) * 512],
                    start=(c == 0),
                    stop=(c == NT - 1),
                )
            nc.scalar.activation(
                dst[t][:, sc * 512 : (sc + 1) * 512],
                ps[:],
                AF.Identity,
                bias=btile[:, t : t + 1],
                scale=s1,
            )

        def qk_chunk(t):
            for w_idx in range(2):
                for sc in range(2):
                    qk_tile(t, w_idx, sc)

        def v_tile(jc, st):
            # one PSUM tile of the V projection; bias added during the
            # PSUM->VZ copy (pre-broadcast bvrB), no ones-row matmul
            ps = psp_pool.tile([P, 512], f32, name="psp")
            for c in range(NT):
                nc.tensor.matmul(
                    ps[:, 0:384],
                    hT[c][:, st * P : (st + 1) * P],
                    wv[c][:, jc * 384 : (jc + 1) * 384],
                    start=(c == 0),
                    stop=(c == NT - 1),
                )
            for hh in range(6):
                i = jc * 6 + hh
                nc.vector.tensor_add(
                    VZ[st][:, i * 65 : i * 65 + 64],
                    ps[:, hh * 64 : (hh + 1) * 64],
                    bvrB[:, jc * 384 + hh * 64 : jc * 384 + (hh + 1) * 64],
                )

        PTS = [None] * NH
        PO = [None] * NH

        def attnv_step(i, kt):
            # two matmuls of head i's attn@V (one per qc), interleaved into
            # the next head's score loop; accumulation groups for the two po
            # banks stay open across interleaved matmuls (skip_group_check)
            if kt == 0:
                PO[i] = [
                    pso_pool.tile([D + 1, 512], f32, name="pso")
                    for _ in range(2)
                ]
            pts = PTS[i]
            for qc in range(2):
                nc.tensor.matmul(
                    PO[i][qc][:],
                    VZ[kt][:, i * 65 : (i + 1) * 65],
                    pts[kt][:, qc * 512 : (qc + 1) * 512],
                    start=(kt == 0),
                    stop=(kt == ST - 1),
                    skip_group_check=True,
                )

        def z_tail(i):
            ch, off = i // 2, (i % 2) * D
            for qc in range(2):
                po = PO[i][qc]
                rz = rz_pool.tile([1, 512], bf16, name="rz")
                with nc.allow_low_precision(reason="1/Z in bf16 for bcast"):
                    nc.vector.reciprocal(rz[:], po[D : D + 1, :])
                # replicate 1/Z across 64 partitions on Pool (SBUF->SBUF),
                # then one DVE mul with po as the lone PSUM operand
                rzb = rz_pool.tile([D, 512], bf16, name="rzb")
                nc.gpsimd.partition_broadcast(rzb[:], rz[:])
                nc.vector.tensor_mul(
                    OUTT[ch][off : off + D, qc * 512 : (qc + 1) * 512],
                    po[0:D, :],
                    rzb[:],
                )

        # ---------------- pipelined projections + attention ----------------
        # only the 12 ones-columns need the memset (strided AP, free=12);
        # the V columns are fully written by the v_tile copies
        for st in range(ST):
            nc.vector.memset(VZ[st][:, D :: D + 1], 1.0)
        qk_chunk(0)
        for i in range(NH):
            ch, off = i // 2, (i % 2) * D
            abig = ABIG[i]
            pts = PTS[i] = [
                pt_pool.tile([P, S], bf16, name=f"pt{kt}") for kt in range(ST)
            ]
            for kt in range(ST):
                ps = pss_pool.tile([P, S], f32, name="pss")
                for qc in range(2):
                    nc.tensor.matmul(
                        ps[:, qc * 512 : (qc + 1) * 512],
                        KT[ch][off : off + D, kt * P : (kt + 1) * P],
                        QT[ch][off : off + D, qc * 512 : (qc + 1) * 512],
                        start=True,
                        stop=True,
                    )
                e = e_pool.tile([P, S], bf16, name="e")
                nc.scalar.activation(e[:], ps[:], AF.Exp)
                t1 = t_pool.tile([P, S], bf16, name="t1")
                # DVE runs bf16 tensor_tensor at 2x (~590ns/tile); Pool's
                # sw path is ~2.2us/tile. 5 of 16 ops per head go to Pool
                # (both engines then finish a head in roughly equal time).
                if kt == 1:
                    nc.gpsimd.tensor_mul(
                        t1[:], e[:], abig[:, kt * S : (kt + 1) * S]
                    )
                else:
                    nc.vector.tensor_mul(
                        t1[:], e[:], abig[:, kt * S : (kt + 1) * S]
                    )
                if kt == 5:
                    nc.gpsimd.tensor_add(pts[kt][:], t1[:], MT[kt])
                else:
                    nc.vector.tensor_add(pts[kt][:], t1[:], MT[kt])
                # interleave projection tiles / previous head's attn@V into
                # this head's score loop so PE never batches a long block
                # between two heads' exp streams
                if i == 0:
                    v_tile(0, kt)
                elif i == 1:
                    v_tile(1, kt)
                elif i in (2, 4, 6, 8) and kt < 4:
                    qk_tile(i // 2 + 1, w_idx=kt // 2, sc=kt % 2)
                if i == 0 and kt >= 4:
                    qk_tile(1, w_idx=(kt - 4) // 2, sc=kt % 2)
                if i >= 1:
                    attnv_step(i - 1, kt)
                # last head: self-interleave its own attn@V two slots behind
                # so only 2 steps remain after the loop (shorter drain)
                if i == NH - 1 and kt >= 2:
                    attnv_step(NH - 1, kt - 2)
            # head i+2's A load reuses head i's buffer slot — issue after
            # head i's last abig read
            if i + 2 < NH:
                issue_a(i + 2, nc.sync)
            if i >= 1:
                z_tail(i - 1)
        attnv_step(NH - 1, ST - 2)
        attnv_step(NH - 1, ST - 1)
        z_tail(NH - 1)
        inner.close()

        # ---------------- output projection ----------------
        with (
            tc.tile_pool(name="res", bufs=4) as res_pool,
            tc.tile_pool(name="psr", bufs=4, space=PSUM) as psr_pool,
        ):
            wo = [woB[:, c * H : (c + 1) * H] for c in range(NT)]
            for st in range(ST):
                res = res_pool.tile([P, H], f32, name="res")
                for jc in range(2):
                    ps = psr_pool.tile([P, 512], f32, name="psr")
                    for ch in range(NT):
                        nc.tensor.matmul(
                            ps[:, 0:384],
                            OUTT[ch][:, st * P : (st + 1) * P],
                            wo[ch][:, jc * 384 : (jc + 1) * 384],
                            start=(ch == 0),
                            stop=(ch == NT - 1),
                        )
                    # bias added during the PSUM->SBUF copy (pre-broadcast)
                    nc.vector.tensor_add(
                        res[:, jc * 384 : (jc + 1) * 384],
                        ps[:, 0:384],
                        borB[:, jc * 384 : (jc + 1) * 384],
                    )
                oeng = nc.sync if st % 2 == 1 else nc.gpsimd
                oeng.dma_start(out_dram[st * P : (st + 1) * P, :], res[:])


@functools.lru_cache(maxsize=1)
def _build():
    from concourse import bacc, tile, mybir

    nc = bacc.Bacc("TRN2", target_bir_lowering=False, debug=False, num_devices=8)
    f32 = mybir.dt.float32
    bf16 = mybir.dt.bfloat16
    dr = {
        "hT": nc.dram_tensor("hT", [H, S], bf16, kind="ExternalInput").ap(),
        "A": nc.dram_tensor("A", [NH, S, S], bf16, kind="ExternalInput").ap(),
        "maskT": nc.dram_tensor("maskT", [S, S], bf16, kind="ExternalInput").ap(),
    }
    for w in ("Wq", "Wk", "Wv", "Wo"):
        dr[w] = nc.dram_tensor(w, [H, H], bf16, kind="ExternalInput").ap()
    for b in ("bq", "bk"):
        dr[b] = nc.dram_tensor(b, [H], f32, kind="ExternalInput").ap()
    for b in ("bv", "bo"):
        dr[b] = nc.dram_tensor(b, [H], bf16, kind="ExternalInput").ap()
    out = nc.dram_tensor("out", [S, H], f32, kind="ExternalOutput").ap()

    with tile.TileContext(nc) as tc:
        _body(nc, tc, tile, mybir, dr, out)
    nc.compile()
    return nc


def make_in_maps(**inputs):
    import ml_dtypes
    bf = ml_dtypes.bfloat16
    h = np.asarray(inputs["h"], np.float32)
    ab = np.asarray(inputs["att_bias"], np.float32)
    mk = np.asarray(inputs["mask"], np.int32)
    shared = {
        "bq": np.asarray(inputs["bq"], np.float32),
        "bk": np.asarray(inputs["bk"], np.float32),
        "bv": np.asarray(inputs["bv"], np.float32).astype(bf),
        "bo": np.asarray(inputs["bo"], np.float32).astype(bf),
    }
    for k in ("Wq", "Wk", "Wv", "Wo"):
        shared[k] = np.asarray(inputs[k], np.float32).astype(bf)
    in_maps = []
    for b in range(8):
        m = dict(shared)
        m["hT"] = np.ascontiguousarray(h[b].T).astype(bf)
        # A[i,k,q] = (1-mask[q,k]) * exp(bias[q,k,i]); maskT[k,q]=mask[q,k]
        mb = mk[b].astype(np.float32)           # [q, k]
        a = np.exp(ab[b]) * (1.0 - mb)[:, :, None]
        m["A"] = np.ascontiguousarray(a.transpose(2, 1, 0)).astype(bf)
        m["maskT"] = np.ascontiguousarray(mb.T).astype(bf)
        in_maps.append(m)
    return in_maps


def kernel(**inputs):
    nc = _build()
    from concourse import bass_utils

    in_maps = make_in_maps(**inputs)
    res = bass_utils.run_bass_kernel_spmd(nc, in_maps, core_ids=list(range(8)))
    return np.stack([r["out"] for r in res.results], axis=0)
